# revision 9
# baseline (speedup 1.0000x reference)
"""Trainium2 Bass kernel for nn_DeepFilter.

Math: the reference unfolds (2I+1)x(2L+1) windows over (freq D, time T) and
does a channel-wise complex-ish multiply-accumulate.  Because input and
filter are shifted by the SAME offset in every tap, the whole thing reduces
to a separable (2I+1)x(2L+1) box filter applied to elementwise product
arrays:

    out_r = Box(xr*fr) - Box(xi*fi)      (the subtract is folded into the
    out_i = 2 * Box(xr*fi)                stage-1 matmul via a negated band)

Per-core layout (pure data parallelism over B across 8 cores):
  - 3 elementwise products on DVE (pr=xr*fr, t2=xi*fi, pi=xr*fi)
  - stage 1 (freq box sum) on PE: psum[t, d_out] = P[d, t] @ Band1[d, d_out]
    accumulating pr@B1 + t2@(-B1) for the real part (output transposed [t,d])
  - stage 2 (time box sum) on PE: psum[d, t_out] = S[t, d] @ Band2[t, t_out]
    (transposes back to [d, t]); the 2x for the imag part is folded into the
    stage-1 PSUM->SBUF copy.
DMA issue is spread over all three DGE paths so transfers overlap: xr/fr on
the SP HWDGE ring, xi/fi on the Pool SWDGE ring, outputs (f16) on the ACT
HWDGE ring.  PSUM->SBUF copies are split between ACT and DVE to balance
engine busy time.  Output is f16 (upcast to f32 on host); bands/consts are
loaded once, outside the timing loop.
"""

import os
import sys

os.environ.setdefault("BASS_NEVER_TRACE", "1")

if "/opt/trn_rl_repo" not in sys.path:
    sys.path.insert(0, "/opt/trn_rl_repo")

import numpy as np

_CACHE = {}
LAST_RESULTS = None

USE_F32R = True
N_CORES = 8
UNROLL = 4          # kernel bodies per For_i trip (reps % UNROLL == 0)
N_S1_DVE = 6        # of the 18 stage-1 PSUM->SBUF copies, run this many on DVE
SWDGE_IN = ("inputs_i", "filters_i")   # input tensors loaded via Pool/SWDGE
SP_IN = ("inputs_r", "filters_r")      # input tensors loaded via SP/HWDGE


def _install_drain_patch():
    """walrus in this env rejects instructions with >2 sem waits; Tile's tail
    drain carries one wait per live proc.  Split them across SP no-ops."""
    import bass_rust
    from concourse import tile as _tile

    if getattr(_tile.TileContext, "_drain_patch_installed", False):
        return

    def _split_drain_and_barrier(self, tick_clock, wait_clock):
        nc = self.nc
        g = tick_clock.global_clock
        vals = list(g)
        n = len(vals)
        for i, v in enumerate(vals):
            if v <= 0:
                continue
            part = bass_rust.VectorClock([v if j == i else 0 for j in range(n)])
            nop = nc.sync.nop(nofuse=True)
            wait_clock.add_sem_waits(nop.ins, bass_rust.ScopedClock({None: part}))
        nc.sync.drain()
        nc.all_engine_barrier()
        assert self.sems is not None
        popped = nc._tile_sem_poison_stack.pop()
        assert popped is self._sem_poison
        nc.clear_and_free_semaphores(list(self.sems.allocated().values()))
        nc.all_engine_barrier()

    _tile.TileContext._drain_and_barrier = _split_drain_and_barrier
    _tile.TileContext._drain_patch_installed = True


_MAX_WAITS = 1


def _split_excess_waits(nc):
    """walrus codegen rejects instructions carrying more than ~2 sem waits.
    Move excess waits onto same-engine no-ops placed just before the
    instruction (engines run their streams in order, so this is
    semantically identical)."""
    from concourse import mybir

    uid = 0
    for fn in nc.m.functions:
        for bb in fn.blocks:
            insts = bb.instructions
            out = []
            changed = False
            for inst in insts:
                si = inst.sync_info
                waits = list(si.on_wait) if si is not None else []
                if len(waits) > _MAX_WAITS:
                    changed = True
                    extra, keep = waits[:-_MAX_WAITS], waits[-_MAX_WAITS:]
                    for i in range(0, len(extra), _MAX_WAITS):
                        chunk = extra[i : i + _MAX_WAITS]
                        nop = mybir.InstNoOp(
                            name=f"wsplit-{uid}", ins=[], outs=[]
                        )
                        uid += 1
                        nop.engine = inst.engine
                        nop.sync_info = mybir.SyncInfo(
                            on_wait=chunk, on_update=[]
                        )
                        out.append(nop)
                    inst.sync_info = mybir.SyncInfo(
                        on_wait=keep, on_update=list(si.on_update)
                    )
                out.append(inst)
            if changed:
                bb.instructions = out


def _build_program(D, T, L, I, reps=1):
    import concourse.bass as bass
    import concourse.tile as tile
    from concourse import mybir

    _install_drain_patch()

    f32 = mybir.dt.float32
    f32r = mybir.dt.float32r
    f16 = mybir.dt.float16
    P = 128
    assert D % P == 0 and D <= 512
    nD = D // P
    TP = T + 2 * L                 # padded time length in SBUF
    W = P - 2 * L                  # complete outputs per stage-2 window
    NW = (T + W - 1) // W          # stage-2 window count
    GW = 4 * W                     # bank group: 4 windows per PSUM bank
    NG = (NW + 3) // 4
    NPROD = 4                      # product compute chunks (c-major interleave)
    NPC = NPROD // nD              # product chunks per d-chunk
    CT = T // NPC                  # columns per product chunk

    unroll = UNROLL if reps > 1 else 1
    assert reps == 1 or reps % unroll == 0

    nc = bass.Bass()
    d_in = {}
    for name in ("inputs_r", "inputs_i", "filters_r", "filters_i"):
        d_in[name] = nc.dram_tensor(name, [D, T], f32, kind="ExternalInput")
    band_dt = f32r if USE_F32R else f32
    d_b1 = nc.dram_tensor("band1", [P, nD, D], band_dt, kind="ExternalInput")
    d_b1n = nc.dram_tensor("band1n", [P, nD, D], band_dt, kind="ExternalInput")
    d_b2 = nc.dram_tensor("band2", [P, 256], f16, kind="ExternalInput")
    d_out = nc.dram_tensor("out", [2 * D, T], f16, kind="ExternalOutput")
    d_cnt = (
        nc.dram_tensor("cnt", [P, 16], f32, kind="ExternalOutput")
        if reps > 1
        else None
    )

    def mm_dt(ap):
        return ap.bitcast(f32r) if USE_F32R else ap

    wr = mm_dt  # producer-side rounding tag for data feeding fp32r matmuls

    import contextlib

    with tile.TileContext(nc) as tc, contextlib.ExitStack() as _stk:
        consts = _stk.enter_context(tc.tile_pool(name="consts", bufs=1))
        ins_pool = _stk.enter_context(tc.tile_pool(name="ins", bufs=1))
        prod_pool = _stk.enter_context(tc.tile_pool(name="prod", bufs=1))
        s_pool = _stk.enter_context(tc.tile_pool(name="s", bufs=6))
        stag_pool = _stk.enter_context(tc.tile_pool(name="stag", bufs=2))
        ps1_pool = _stk.enter_context(tc.tile_pool(name="ps1", bufs=4, space="PSUM"))
        ps2_pool = _stk.enter_context(tc.tile_pool(name="ps2", bufs=1, space="PSUM"))

        # ---- constants + persistent tiles, loaded/zeroed ONCE ----
        b1 = consts.tile([P, nD, D], band_dt, tag="b1")
        b1n = consts.tile([P, nD, D], band_dt, tag="b1n")
        b2 = consts.tile([P, 256], f16, tag="b2")
        nc.sync.dma_start(out=b1, in_=d_b1[:, :, :])
        nc.sync.dma_start(out=b1n, in_=d_b1n[:, :, :])
        nc.sync.dma_start(out=b2, in_=d_b2[:, :])
        cnt_t = None
        if d_cnt is not None:
            cnt_t = consts.tile([P, 16], f32, tag="cnt")
            nc.vector.memset(cnt_t, 0.0)

        sb_in = {}
        for name in d_in:
            sb_in[name] = ins_pool.tile([P, nD, T], f32, tag=name, name=name)
        # product tiles live in padded time coords; pad columns stay zero
        # across loop iterations (products only ever write cols [L, L+T)).
        pr = prod_pool.tile([P, nD, TP], f32, tag="pr")
        pi = prod_pool.tile([P, nD, TP], f32, tag="pi")
        t2 = prod_pool.tile([P, nD, TP], f32, tag="t2")
        for pt in (pr, pi, t2):
            for c in range(nD):
                for side in (0, TP - L):
                    nc.vector.memset(pt[:, c, side : side + L], 0.0)
        stags = [
            stag_pool.tile([P, nD, T], f16, tag=f"stag{comp}", name=f"stag{comp}")
            for comp in range(2)
        ]

        if reps > 1:
            _stk.enter_context(tc.For_i(0, reps // unroll, 1))

        for _body in range(unroll):
            # ---- input DMA: xr/fr on the SP ring, xi/fi on Pool/SWDGE ----
            # 512 KB chunks ([128, 1024] cols), interleaved across the two
            # tensors of each ring and across d-chunks (c) so products (which
            # need all four tensors for one (c, j) tile) unblock earliest.
            for j in range(NPC):
                for c in range(nD):
                    cs = slice(j * CT, (j + 1) * CT)
                    for name in SP_IN:
                        src = d_in[name][:, :].rearrange("(c p) t -> p c t", p=P)
                        nc.sync.dma_start(
                            out=sb_in[name][:, c, cs], in_=src[:, c, cs]
                        )
                    for name in SWDGE_IN:
                        src = d_in[name][:, :].rearrange("(c p) t -> p c t", p=P)
                        nc.gpsimd.dma_start(
                            out=sb_in[name][:, c, cs], in_=src[:, c, cs]
                        )

            xr, xi = sb_in["inputs_r"], sb_in["inputs_i"]
            fr, fi = sb_in["filters_r"], sb_in["filters_i"]

            # ---- products on DVE: pr=xr*fr, t2=xi*fi, pi=xr*fi ----
            # 2D contiguous [128, CT] slices (3D APs are ~2.5x slower on DVE)
            xr_f = xr.rearrange("p c t -> p (c t)")
            xi_f = xi.rearrange("p c t -> p (c t)")
            fr_f = fr.rearrange("p c t -> p (c t)")
            fi_f = fi.rearrange("p c t -> p (c t)")
            pr_f = pr.rearrange("p c t -> p (c t)")
            pi_f = pi.rearrange("p c t -> p (c t)")
            t2_f = t2.rearrange("p c t -> p (c t)")
            for k in range(NPROD):
                j, c = k // nD, k % nD     # match DMA arrival order (j-major)
                a0 = c * T + j * CT        # input flat col
                b0 = c * TP + L + j * CT   # product flat col
                ics = slice(a0, a0 + CT)
                pcs = slice(b0, b0 + CT)
                nc.vector.tensor_mul(wr(pi_f[:, pcs]), xr_f[:, ics], fi_f[:, ics])
                nc.vector.tensor_mul(wr(pr_f[:, pcs]), xr_f[:, ics], fr_f[:, ics])
                nc.vector.tensor_mul(wr(t2_f[:, pcs]), xi_f[:, ics], fi_f[:, ics])

            # ---- two-stage banded-matmul box filter ----
            # real part accumulates pr@B1 + t2@(-B1) in stage-1 PSUM; imag is
            # pi@B1 with the 2x folded into the PSUM->SBUF copy.  stage-1 psum
            # banks hold a PAIR of windows (cols 0:256 / 256:512) so the
            # PSUM->SBUF copy is one big op per pair.
            comps = ((pr, 1.0), (pi, 2.0))
            s1_idx = 0
            for g in range(NG):
                ws = list(range(4 * g, min(4 * g + 4, NW)))
                ps2 = [
                    ps2_pool.tile(
                        [P, nD, 512], f32, tag=f"ps2_{comp}", name=f"ps2_{comp}"
                    )
                    for comp in range(2)
                ]
                for pair_i in range(0, len(ws), 2):
                    pair = ws[pair_i : pair_i + 2]
                    for comp, (pt, scale) in enumerate(comps):
                        ps1 = ps1_pool.tile([P, 512], f32, tag="ps1", name="ps1")
                        srcs = [(pt, b1)] if comp == 1 else [(pr, b1), (t2, b1n)]
                        nsrc = len(srcs)
                        for j, w in enumerate(pair):
                            M = min(P, T + 2 * L - w * W)
                            for c in range(nD):
                                for si, (st, bb) in enumerate(srcs):
                                    nc.tensor.matmul(
                                        ps1[0:M, j * D : (j + 1) * D],
                                        mm_dt(st[:, c, w * W : w * W + M]),
                                        bb[:, c, :],
                                        start=(j == 0 and c == 0 and si == 0),
                                        stop=(
                                            w == pair[-1]
                                            and c == nD - 1
                                            and si == nsrc - 1
                                        ),
                                    )
                        sw = s_pool.tile([P, 512], f16, tag="sw", name="sw")
                        Mmax = min(P, T + 2 * L - pair[0] * W)
                        span = len(pair) * D
                        on_dve = (
                            ((s1_idx + 1) * N_S1_DVE) // 18
                            > (s1_idx * N_S1_DVE) // 18
                        )
                        s1_idx += 1
                        if scale == 1.0:
                            if on_dve:
                                nc.vector.tensor_copy(
                                    sw[0:Mmax, 0:span], ps1[0:Mmax, 0:span]
                                )
                            else:
                                nc.scalar.copy(
                                    sw[0:Mmax, 0:span], ps1[0:Mmax, 0:span]
                                )
                        else:
                            if on_dve:
                                nc.vector.tensor_scalar_mul(
                                    sw[0:Mmax, 0:span], ps1[0:Mmax, 0:span], scale
                                )
                            else:
                                nc.scalar.mul(
                                    sw[0:Mmax, 0:span], ps1[0:Mmax, 0:span], scale
                                )
                        for j, w in enumerate(pair):
                            s = w - 4 * g
                            M = min(P, T + 2 * L - w * W)
                            for c in range(nD):
                                nc.tensor.matmul(
                                    ps2[comp][:, c, s * W : s * W + W],
                                    sw[0:M, j * D + c * P : j * D + (c + 1) * P],
                                    b2[0:M, 0:W],
                                    start=(s == 0),
                                    stop=(w == ws[-1]),
                                )
                cw = min(GW, T - g * GW)
                for comp in range(2):
                    nc.scalar.copy(
                        stags[comp][:, :, g * GW : g * GW + cw],
                        ps2[comp][:, :, 0:cw],
                    )
                # ship completed output pieces (ACT ring) while the rest
                # computes; piece boundaries: g=1 -> [0, 2GW), g=3 ->
                # [2GW, 4GW), g=NG-1 -> [4GW, T)
                piece = (
                    slice(0, 2 * GW) if g == 1
                    else slice(2 * GW, 4 * GW) if g == 3
                    else slice(4 * GW, T) if g == NG - 1
                    else None
                )
                if piece is not None:
                    for comp in range(2):
                        dst = d_out[comp * D : (comp + 1) * D, :].rearrange(
                            "(c p) t -> p c t", p=P
                        )
                        eng = nc.sync if comp == 0 else nc.gpsimd
                        eng.dma_start(
                            out=dst[:, :, piece], in_=stags[comp][:, :, piece]
                        )
            if d_cnt is not None:
                nc.vector.tensor_scalar_add(cnt_t, cnt_t, 1.0)
        if d_cnt is not None:
            nc.sync.dma_start(out=d_cnt[:, :], in_=cnt_t)

    _split_excess_waits(nc)
    # populate .instr bytes for extended-inst InstISA subclasses (the loop's
    # SWDGE sem reset lowers to one); walrus rejects empty instr bytes with
    # "ISA wrong length"
    mybir.codegen_inst_isa_subclasses(nc)
    return nc


def _get_program(D, T, L, I, reps=1):
    key = (D, T, L, I, USE_F32R, reps, UNROLL, N_S1_DVE)
    if key not in _CACHE:
        _CACHE[key] = _build_program(D, T, L, I, reps)
    return _CACHE[key]


_RUNNER_CACHE = {}


def _get_runner(nc, n_cores):
    """Persistent jitted executor for `nc` (run_bass_via_pjrt re-traces on
    every call, costing ~2s; this caches the jax.jit so repeat kernel()
    calls only pay transfer + execute)."""
    key = (id(nc), n_cores)
    if key in _RUNNER_CACHE:
        return _RUNNER_CACHE[key]

    import jax
    from jax.experimental.shard_map import shard_map
    from jax.sharding import Mesh, PartitionSpec

    from concourse import bass2jax, mybir

    bass2jax.install_neuronx_cc_hook()
    partition_name = (
        nc.partition_id_tensor.name if nc.partition_id_tensor else None
    )
    in_names, out_names, out_avals, out_shapes = [], [], [], []
    for alloc in nc.m.functions[0].allocations:
        if not isinstance(alloc, mybir.MemoryLocationSet):
            continue
        name = alloc.memorylocations[0].name
        if alloc.kind == "ExternalInput":
            if name != partition_name:
                in_names.append(name)
        elif alloc.kind == "ExternalOutput":
            shape = tuple(alloc.tensor_shape)
            dtype = mybir.dt.np(alloc.dtype)
            out_names.append(name)
            out_avals.append(jax.core.ShapedArray(shape, dtype))
            out_shapes.append((shape, dtype))
    n_params = len(in_names)
    all_names = in_names + out_names
    if partition_name is not None:
        all_names.append(partition_name)
    donate = tuple(range(n_params, n_params + len(out_names)))

    def _body(*args):
        operands = list(args)
        if partition_name is not None:
            operands.append(bass2jax.partition_id_tensor())
        outs = bass2jax._bass_exec_p.bind(
            *operands,
            out_avals=tuple(out_avals),
            in_names=tuple(all_names),
            out_names=tuple(out_names),
            lowering_input_output_aliases=(),
            sim_require_finite=True,
            sim_require_nnan=True,
            nc=nc,
        )
        return tuple(outs)

    devices = jax.devices()[:n_cores]
    mesh = Mesh(np.asarray(devices), ("core",))
    in_specs = (PartitionSpec("core"),) * (n_params + len(out_names))
    out_specs = (PartitionSpec("core"),) * len(out_names)
    sharded = jax.jit(
        shard_map(
            _body, mesh=mesh, in_specs=in_specs, out_specs=out_specs,
            check_rep=False,
        ),
        donate_argnums=donate,
        keep_unused=True,
    )

    def run(in_maps):
        n = len(in_maps)
        assert n == n_cores
        concat_in = [
            np.concatenate([np.asarray(m[nm]) for m in in_maps], axis=0)
            for nm in in_names
        ]
        zeros = [
            np.zeros((n * s[0], *s[1:]), dt) for (s, dt) in out_shapes
        ]
        outs = sharded(*concat_in, *zeros)
        return [
            {
                nm: np.asarray(outs[i]).reshape(n, *out_shapes[i][0])[c]
                for i, nm in enumerate(out_names)
            }
            for c in range(n)
        ]

    _RUNNER_CACHE[key] = run
    return run


def _bands(D, T, L, I):
    P = 128
    nD = D // P
    band1 = np.zeros((P, nD, D), dtype=np.float32)
    for c in range(nD):
        for k in range(P):
            d_in = c * P + k
            lo = max(0, d_in - I)
            hi = min(D - 1, d_in + I)
            band1[k, c, lo : hi + 1] = 1.0
    W = P - 2 * L
    band2 = np.zeros((P, 256), dtype=np.float16)
    for k in range(P):
        lo = max(0, k - 2 * L)
        hi = min(W - 1, k)
        if lo <= hi:
            band2[k, lo : hi + 1] = 1.0
    return band1, band2


def kernel(inputs_r, inputs_i, filters_r, filters_i, L, I):
    global LAST_RESULTS
    from concourse.bass_utils import run_bass_kernel_spmd

    L = int(L)
    I = int(I)
    xr = np.ascontiguousarray(np.asarray(inputs_r), dtype=np.float32)
    xi = np.ascontiguousarray(np.asarray(inputs_i), dtype=np.float32)
    fr = np.ascontiguousarray(np.asarray(filters_r), dtype=np.float32)
    fi = np.ascontiguousarray(np.asarray(filters_i), dtype=np.float32)
    B, D, T = xr.shape

    nc = _get_program(D, T, L, I)
    band1, band2 = _bands(D, T, L, I)

    outs = []
    step = min(B, N_CORES)
    for s in range(0, B, step):
        batch = list(range(s, min(s + step, B)))
        in_maps = [
            {
                "inputs_r": xr[b],
                "inputs_i": xi[b],
                "filters_r": fr[b],
                "filters_i": fi[b],
                "band1": band1,
                "band1n": -band1,
                "band2": band2,
            }
            for b in batch
        ]
        try:
            runner = _get_runner(nc, len(batch))
            results = runner(in_maps)
        except Exception:
            results = run_bass_kernel_spmd(
                nc, in_maps, core_ids=list(range(len(batch)))
            ).results
        LAST_RESULTS = results
        outs.extend(
            results[i]["out"].astype(np.float32) for i in range(len(batch))
        )
    return np.stack(outs, axis=0)
